# revision 1
# baseline (speedup 1.0000x reference)
"""Trainium2 Bass kernel for BroadcastingSelfAttention.

Reference computation (see problem):
    score(s,b,t) = softplus(sum_f X[s,b,f,t] * W[s,f] + bias[s])
    w(d,s,b,t)   = softmax_s(-score(s,b,t) * dist(d,s))
    out(d,b,f,t) = sum_s w(d,s,b,t) * X[s,b,f,t]

Shapes: S=64, B=16, F=64, T=96, D=1024 (= 32*32 target grid).

Sharding: B=16 split across 8 cores (2 batches per core). Each core reads its
X slice (3.1 MB) + full dist/params, writes its out slice (50 MB).

Per-core dataflow (per batch b, per t-pair):
  * e2[(th,s)=128p, d=1024] = exp(score(s, 2*tp+th) * (-dist(d,s)))  -- one ACT op
    (t-parity packs two t's into 128 partitions; dist pre-negated so the ACT
    per-partition `scale` operand carries +score)
  * 16 matmuls: stationary e2[s=64, d_blk=128 cols], moving [X[s,f,t] | ones]
    (N=65; fused denominator column), row-tiled by t-parity (tile_position 0/64)
    -> psum[d=128, f'=65] per (parity, d_blk)
  * reciprocal of the 16 denominator columns (DVE)
  * drain: TT multiply psum * recip (free-dim step-0 broadcast) -> staging SBUF
    in d-partition layout; t accumulates in stage; DMA out 128B-contiguous runs.
"""

import numpy as np

import concourse.bass as bass
import concourse.tile as tile
from concourse import bacc, mybir
from concourse import bass_utils

F32 = mybir.dt.float32
BF16 = mybir.dt.bfloat16

# Problem shapes (hardcoded per contract)
S = 64          # sources
B = 16          # total batch
NCORES = 8
BL = B // NCORES  # batches per core = 2
F = 64          # features
T = 96          # time
D = 1024        # flattened target grid 32*32
DBLK = D // 128  # 8 d-blocks of 128
TP = T // 2     # 48 t-pairs
TCH = 32        # t-chunk (stage tile holds 32 t values = 16 pairs)
NCH = T // TCH  # 3 chunks
RPC = TCH // 2  # 16 rounds (t-pairs) per chunk

FT = F * T            # 6144
SSTRIDE = BL * F * T  # x: s stride, 12288
OSTRIDE = BL * F * T  # out: d stride, 12288

# MM_DT: dtype of matmul operands (e2 weights + moving X). fp32 is exact;
# bf16 halves weight-load time (FWL) at ~0.4% relative error.
MM_DT = BF16
# OUT_DT: dtype of the staged/DMA'd output (host upcasts to f32). bf16 halves
# the dominant output DMA traffic at ~0.4% relative error.
OUT_DT = BF16
# Every Nth round bounces psum through SBUF via ScalarE-copy + GpSimd-
# normalize instead of the VectorE drain (0 = never). Offloads the DVE
# bottleneck without extra psum pressure.
BOUNCE_PERIOD = 4


def build_kernel():
    nc = bacc.Bacc("TRN2", target_bir_lowering=False, debug=False,
                   num_devices=NCORES)

    # xp[th, s, b, f, tp] = X[s, b, f, 2*tp+th]  (host pre-shuffled t-parity)
    x_t = nc.dram_tensor("xp", (2, S, BL, F, TP), F32, kind="ExternalInput")
    # ndist_T[s, d] = -dist[d, s]  (host pre-transposed + negated)
    dist_t = nc.dram_tensor("ndist_T", (S, D), F32, kind="ExternalInput")
    w_t = nc.dram_tensor("w", (S, F), F32, kind="ExternalInput")
    bias_t = nc.dram_tensor("bias", (S, 1), F32, kind="ExternalInput")
    # Output in hardware-native layout: one fully-contiguous run per
    # partition per DMA (host un-permutes). Index: [b, ch, dblk, tlh, p, f, tlo]
    # -> out[dblk*128+p, b, f, ch*TCH + tlh*(TCH//2) + tlo].
    out_t = nc.dram_tensor("out_hw", (BL, NCH, DBLK, 4, 128, F, TCH // 4),
                           OUT_DT, kind="ExternalOutput")

    def dram_ap(t, offset, ap):
        base = t.ap()
        return bass.AP(tensor=base.tensor, offset=offset, ap=ap)

    with tile.TileContext(nc) as tc:
        with (
            tc.tile_pool(name="statics", bufs=1) as statics,
            tc.tile_pool(name="xin", bufs=2) as xin,
            tc.tile_pool(name="xmm", bufs=2) as xmm,
            tc.tile_pool(name="score", bufs=2) as scorep,
            tc.tile_pool(name="e2p", bufs=4) as e2p,
            tc.tile_pool(name="stage", bufs=3 if OUT_DT is BF16 else 2) as stagep,
            tc.tile_pool(name="small", bufs=4) as small,
            tc.tile_pool(name="psum", bufs=2, space="PSUM") as psump,
        ):
            # ---- static tiles -------------------------------------------------
            # ndist2[(th,s)=128p, d] = -dist(d,s), replicated across t-parity
            ndist2 = statics.tile([128, D], F32)
            for th in range(2):
                nc.gpsimd.dma_start(
                    out=ndist2[th * S : (th + 1) * S, :],
                    in_=dram_ap(dist_t, 0, [[D, S], [1, D]]),
                )

            # w2[(th,s), f] = W[s,f]
            w2 = statics.tile([128, F], F32)
            for th in range(2):
                nc.gpsimd.dma_start(
                    out=w2[th * S : (th + 1) * S, :],
                    in_=dram_ap(w_t, 0, [[F, S], [1, F]]),
                )
            # bias2[(th,s), 1]
            bias2 = statics.tile([128, 1], F32)
            for th in range(2):
                nc.gpsimd.dma_start(
                    out=bias2[th * S : (th + 1) * S, :],
                    in_=dram_ap(bias_t, 0, [[1, S], [0, 1]]),
                )

            for b in range(BL):
                # ---- x2[(th,s)=128p, f'=65, tp=48], f'==64 is the ones column
                x2 = xin.tile([128, F + 1, TP], F32)
                for th in range(2):
                    # parity halves on different DMA paths (SWDGE + HWDGE);
                    # b=0 th=0 rides the idle ACT HWDGE queue (startup path)
                    eng = (nc.scalar if b == 0 else nc.gpsimd) if th == 0 else nc.sync
                    eng.dma_start(
                        out=x2[th * S : (th + 1) * S, 0:F, :],
                        in_=dram_ap(
                            x_t, th * (S * BL * F * TP) + b * (F * TP),
                            [[BL * F * TP, S], [TP, F], [1, TP]],
                        ),
                    )
                nc.vector.memset(x2[:, F : F + 1, :], 1.0)

                if MM_DT is F32:
                    x2m = x2
                else:
                    x2m = xmm.tile([128, F + 1, TP], MM_DT)
                    nc.gpsimd.tensor_copy(x2m[:], x2[:])

                # ---- score_t[(th,s), tp] = softplus(sum_f x*w + bias) --------
                ztmp = scorep.tile([128, TP, F], F32, tag="ztmp")
                nc.vector.tensor_tensor(
                    out=ztmp[:],
                    in0=x2[:, 0:F, :].rearrange("p f t -> p t f"),
                    in1=w2[:].unsqueeze(1).broadcast_to([128, TP, F]),
                    op=mybir.AluOpType.mult,
                )
                z = scorep.tile([128, TP], F32, tag="z")
                nc.vector.reduce_sum(out=z[:], in_=ztmp[:],
                                     axis=mybir.AxisListType.X)
                # softplus(z+bias) = ln(1 + exp(z+bias)); exp & ln share one
                # activation table (natural_log_exp_and_others), softplus does
                # not exist on cayman hardware tables.
                ez = scorep.tile([128, TP], F32, tag="ez")
                nc.scalar.activation(
                    out=ez[:], in_=z[:],
                    func=mybir.ActivationFunctionType.Exp,
                    bias=bias2[:, 0:1], scale=1.0,
                )
                nc.vector.tensor_scalar_add(ez[:], ez[:], 1.0)
                score_t = scorep.tile([128, TP], F32, tag="score")
                nc.scalar.activation(
                    out=score_t[:], in_=ez[:],
                    func=mybir.ActivationFunctionType.Ln,
                )

                for ch in range(NCH):
                    # stage[(d%128)=128p, dh=2, dl=4, tlh=2, f=64, tlo=16]
                    # (tl = tlh*16 + tlo; the tlh split lets the first half-
                    # chunk DMA out while the second half is still draining)
                    stage = stagep.tile([128, 2, 4, 4, F, TCH // 4], OUT_DT)
                    for r in range(RPC):
                        tp = ch * RPC + r
                        # e2[(th,s), d] = exp(score * -dist)
                        e2 = e2p.tile([128, D], MM_DT)
                        nc.scalar.activation(
                            out=e2[:], in_=ndist2[:],
                            func=mybir.ActivationFunctionType.Exp,
                            scale=score_t[:, tp : tp + 1],
                        )

                        # pm[128, 2048]: parity p-> cols p*1024; within parity:
                        # dblk = dh*4+dl -> col dh*512 + dl*65 (65 cols each)
                        pm = psump.tile([128, 2, 2, 512], F32, tag="pm")
                        for par in range(2):
                            p0 = par * S
                            for dh in range(2):
                                for dl in range(4):
                                    dblk = dh * 4 + dl
                                    nc.tensor.matmul(
                                        out=pm[:, par, dh, dl * 65 : dl * 65 + 65],
                                        lhsT=e2[p0 : p0 + S,
                                                dblk * 128 : (dblk + 1) * 128],
                                        rhs=x2m[p0 : p0 + S, :, tp],
                                        start=True, stop=True,
                                    )

                        # reciprocal of the 16 denominator columns
                        rc = small.tile([128, 2, 2, 4], F32, tag="rc")
                        nc.vector.reciprocal(
                            out=rc[:],
                            in_=pm[:, :, :, 64 : 64 + 3 * 65 + 1 : 65],
                        )

                        # drain + normalize: stage[.., 2r+par] = pm * rc
                        # (one rank-4 TT covers both parities)
                        tlh, tlo = (2 * r) // 8, (2 * r) % 8
                        out_ap = stage[:, :, :, tlh, :,
                                       tlo : tlo + 2].rearrange(
                            "p a c e t -> p t a c e")
                        rc_b = rc[:].unsqueeze(4).broadcast_to([128, 2, 2, 4, F])
                        rg = (b * NCH + ch) * RPC + r
                        if BOUNCE_PERIOD and rg % BOUNCE_PERIOD == (
                                BOUNCE_PERIOD - 1):
                            # psum -> SBUF on ScalarE, normalize on GpSimd
                            tmp = small.tile([128, 2, 2, 260], F32, tag="bnc")
                            nc.scalar.activation(
                                out=tmp[:], in_=pm[:, :, :, 0:260],
                                func=mybir.ActivationFunctionType.Copy,
                            )
                            nc.gpsimd.tensor_tensor(
                                out=out_ap,
                                in0=tmp[:].rearrange(
                                    "p a h (c e) -> p a h c e",
                                    c=4)[:, :, :, :, 0:F],
                                in1=rc_b,
                                op=mybir.AluOpType.mult,
                            )
                        else:
                            nc.vector.tensor_tensor(
                                out=out_ap,
                                in0=pm[:, :, :, 0 : 4 * 65].rearrange(
                                    "p a h (c e) -> p a h c e",
                                    c=4)[:, :, :, :, 0:F],
                                in1=rc_b,
                                op=mybir.AluOpType.mult,
                            )

                    # ---- DMA chunk out (one contiguous run per partition per
                    # half-chunk; first half overlaps second half's drains) --
                    hp = F * (TCH // 4)  # 512 elements per partition
                    for tlh in range(4):
                        for dh in range(2):
                            for dl in range(4):
                                dblk = dh * 4 + dl
                                nc.sync.dma_start(
                                    out=dram_ap(
                                        out_t,
                                        (((b * NCH + ch) * DBLK + dblk) * 4
                                         + tlh) * 128 * hp,
                                        [[hp, 128], [1, hp]],
                                    ),
                                    in_=stage[:, dh, dl, tlh, :, :],
                                )

    nc.compile()
    return nc


_NC_CACHE = None


def _get_nc():
    global _NC_CACHE
    if _NC_CACHE is None:
        _NC_CACHE = build_kernel()
    return _NC_CACHE


def kernel(X, dist, attention_weight, attention_bias):
    X = np.asarray(X, dtype=np.float32)                                # (S,B,F,T)
    dist_np = np.asarray(dist, dtype=np.float32).reshape(-1, S)        # (D,S)
    ndist_T = np.ascontiguousarray(-dist_np.T)                         # (S,D)
    w_np = np.ascontiguousarray(np.asarray(attention_weight, np.float32))
    bias_np = np.ascontiguousarray(
        np.asarray(attention_bias, np.float32).reshape(S, 1))
    # xp[th, s, b, f, tp] = X[s, b, f, 2*tp+th]
    xp_full = np.ascontiguousarray(
        X.reshape(S, B, F, TP, 2).transpose(4, 0, 1, 2, 3))

    nc = _get_nc()
    in_maps = []
    for c in range(NCORES):
        in_maps.append({
            "xp": np.ascontiguousarray(xp_full[:, :, c * BL : (c + 1) * BL]),
            "ndist_T": ndist_T,
            "w": w_np,
            "bias": bias_np,
        })
    res = bass_utils.run_bass_kernel_spmd(nc, in_maps, core_ids=list(range(NCORES)))
    # out_hw[b, ch, dblk, tlh, p, f, tlo]
    #   -> out[dblk*128+p, B-global, f, ch*TCH + tlh*16 + tlo]
    out = np.empty((D, B, F, T), dtype=np.float32)
    for c in range(NCORES):
        hw = res.results[c]["out_hw"]                # (BL,NCH,8,2,128,64,16)
        # -> (dblk, p, b, f, ch, tlh, tlo)
        out[:, c * BL : (c + 1) * BL] = (
            hw.astype(np.float32)
            .transpose(2, 4, 0, 5, 1, 3, 6)
            .reshape(D, BL, F, T)
        )
    return out.reshape(32, 32, B, F, T)



# revision 10
# speedup vs baseline: 2.2868x; 2.2868x over previous
"""Trainium2 Bass kernel for BroadcastingSelfAttention.

Reference computation (see problem):
    score(s,b,t) = softplus(sum_f X[s,b,f,t] * W[s,f] + bias[s])
    w(d,s,b,t)   = softmax_s(-score(s,b,t) * dist(d,s))
    out(d,b,f,t) = sum_s w(d,s,b,t) * X[s,b,f,t]

Shapes: S=64, B=16, F=64, T=96, D=1024 (= 32*32 target grid).

Sharding: B=16 split across 8 cores (2 batches per core). Each core reads its
X slice (3.1 MB) + full dist/params, writes its out slice (50 MB).

Per-core dataflow (per batch b, per t-pair):
  * e2[(th,s)=128p, d=1024] = exp(score(s, 2*tp+th) * (-dist(d,s)))  -- one ACT op
    (t-parity packs two t's into 128 partitions; dist pre-negated so the ACT
    per-partition `scale` operand carries +score)
  * 16 matmuls: stationary e2[s=64, d_blk=128 cols], moving [X[s,f,t] | ones]
    (N=65; fused denominator column), row-tiled by t-parity (tile_position 0/64)
    -> psum[d=128, f'=65] per (parity, d_blk)
  * reciprocal of the 16 denominator columns (DVE)
  * drain: TT multiply psum * recip (free-dim step-0 broadcast) -> staging SBUF
    in d-partition layout; t accumulates in stage; DMA out 128B-contiguous runs.
"""

import numpy as np

import concourse.bass as bass
import concourse.tile as tile
from concourse import bacc, mybir
from concourse import bass_utils

F32 = mybir.dt.float32
BF16 = mybir.dt.bfloat16

# Problem shapes (hardcoded per contract)
S = 64          # sources
B = 16          # total batch
NCORES = 8
BL = B // NCORES  # batches per core = 2
F = 64          # features
T = 96          # time
D = 1024        # flattened target grid 32*32
DBLK = D // 128  # 8 d-blocks of 128
TP = T // 2     # 48 t-pairs
TCH = 32        # t-chunk (stage tile holds 32 t values = 16 pairs)
NCH = T // TCH  # 3 chunks
RPC = TCH // 2  # 16 rounds (t-pairs) per chunk

FT = F * T            # 6144
SSTRIDE = BL * F * T  # x: s stride, 12288
OSTRIDE = BL * F * T  # out: d stride, 12288

# MM_DT: dtype of matmul operands (e2 weights + moving X). fp32 is exact;
# bf16 halves weight-load time (FWL) at ~0.4% relative error.
MM_DT = BF16
# OUT_DT: dtype of the staged/DMA'd output (host upcasts to f32). bf16 halves
# the dominant output DMA traffic at ~0.4% relative error.
OUT_DT = BF16
# Every Nth round bounces psum through SBUF via ScalarE-copy + GpSimd-
# normalize instead of the VectorE drain (0 = never). Offloads the DVE
# bottleneck without extra psum pressure.
BOUNCE_PERIOD = 4


def build_kernel():
    nc = bacc.Bacc("TRN2", target_bir_lowering=False, debug=False,
                   num_devices=NCORES)

    # xp[th, s, b, tp, f] = X[s, b, f, 2*tp+th]  (host pre-shuffled t-parity,
    # t-major so SBUF reads are f-contiguous)
    x_t = nc.dram_tensor("xp", (2, S, BL, TP, F), F32, kind="ExternalInput")
    # ndist_T[s, d] = -dist[d, s]  (host pre-transposed + negated)
    dist_t = nc.dram_tensor("ndist_T", (S, D), F32, kind="ExternalInput")
    w_t = nc.dram_tensor("w", (S, F), F32, kind="ExternalInput")
    bias_t = nc.dram_tensor("bias", (S, 1), F32, kind="ExternalInput")
    # Output in hardware-native layout: one fully-contiguous run per
    # partition per DMA (host un-permutes). Index: [b, ch, tlh, dblk, p, tlo, f]
    # -> out[dblk*128+p, b, f, ch*TCH + tlh*(TCH//4) + tlo].
    out_t = nc.dram_tensor("out_hw", (BL, NCH, 4, DBLK, 128, TCH // 4, F),
                           OUT_DT, kind="ExternalOutput")

    def dram_ap(t, offset, ap):
        base = t.ap()
        return bass.AP(tensor=base.tensor, offset=offset, ap=ap)

    with tile.TileContext(nc) as tc:
        with (
            tc.tile_pool(name="statics", bufs=1) as statics,
            tc.tile_pool(name="xin", bufs=2) as xin,
            tc.tile_pool(name="xmm", bufs=2) as xmm,
            tc.tile_pool(name="score", bufs=2) as scorep,
            tc.tile_pool(name="e2p", bufs=4) as e2p,
            tc.tile_pool(name="stage", bufs=3 if OUT_DT is BF16 else 2) as stagep,
            tc.tile_pool(name="small", bufs=4) as small,
            tc.tile_pool(name="psum", bufs=2, space="PSUM") as psump,
        ):
            # ---- static tiles -------------------------------------------------
            # ndist2[(th,s)=128p, d] = -dist(d,s), replicated across t-parity
            ndist2 = statics.tile([128, D], F32)
            for th in range(2):
                nc.gpsimd.dma_start(
                    out=ndist2[th * S : (th + 1) * S, :],
                    in_=dram_ap(dist_t, 0, [[D, S], [1, D]]),
                )

            # w2[(th,s), f] = W[s,f]
            w2 = statics.tile([128, F], F32)
            for th in range(2):
                nc.gpsimd.dma_start(
                    out=w2[th * S : (th + 1) * S, :],
                    in_=dram_ap(w_t, 0, [[F, S], [1, F]]),
                )
            # bias2[(th,s), 1]
            bias2 = statics.tile([128, 1], F32)
            for th in range(2):
                nc.gpsimd.dma_start(
                    out=bias2[th * S : (th + 1) * S, :],
                    in_=dram_ap(bias_t, 0, [[1, S], [0, 1]]),
                )

            for b in range(BL):
                # ---- x2[(th,s)=128p, tp=48, f'=65], f'==64 is the ones column
                x2 = xin.tile([128, TP, F + 1], F32)
                for th in range(2):
                    # parity halves on different DMA paths (SWDGE + HWDGE);
                    # b=0 th=0 rides the idle ACT HWDGE queue (startup path)
                    eng = (nc.scalar if b == 0 else nc.gpsimd) if th == 0 else nc.sync
                    eng.dma_start(
                        out=x2[th * S : (th + 1) * S, :, 0:F],
                        in_=dram_ap(
                            x_t, th * (S * BL * F * TP) + b * (F * TP),
                            [[BL * F * TP, S], [F, TP], [1, F]],
                        ),
                    )
                nc.vector.memset(x2[:, :, F : F + 1], 1.0)

                if MM_DT is F32:
                    x2m = x2
                else:
                    x2m = xmm.tile([128, TP, F + 1], MM_DT)
                    nc.gpsimd.tensor_copy(x2m[:], x2[:])

                # ---- score_t[(th,s), tp] = softplus(sum_f x*w + bias) --------
                ztmp = scorep.tile([128, TP, F], F32, tag="ztmp")
                nc.vector.tensor_tensor(
                    out=ztmp[:],
                    in0=x2[:, :, 0:F],
                    in1=w2[:].unsqueeze(1).broadcast_to([128, TP, F]),
                    op=mybir.AluOpType.mult,
                )
                z = scorep.tile([128, TP], F32, tag="z")
                nc.vector.reduce_sum(out=z[:], in_=ztmp[:],
                                     axis=mybir.AxisListType.X)
                # softplus(z+bias) = ln(1 + exp(z+bias)); exp & ln share one
                # activation table (natural_log_exp_and_others), softplus does
                # not exist on cayman hardware tables.
                ez = scorep.tile([128, TP], F32, tag="ez")
                nc.scalar.activation(
                    out=ez[:], in_=z[:],
                    func=mybir.ActivationFunctionType.Exp,
                    bias=bias2[:, 0:1], scale=1.0,
                )
                nc.vector.tensor_scalar_add(ez[:], ez[:], 1.0)
                score_t = scorep.tile([128, TP], F32, tag="score")
                nc.scalar.activation(
                    out=score_t[:], in_=ez[:],
                    func=mybir.ActivationFunctionType.Ln,
                )

                for ch in range(NCH):
                    # stage[(d%128)=128p, dh=2, dl=4, tlh=4, tlo=8, f=64]
                    # (t_local = tlh*8 + tlo; f innermost so drains write
                    # contiguous 128B runs; the tlh split lets early quarter-
                    # chunks DMA out while later rounds are still draining)
                    stage = stagep.tile([128, 2, 4, 4, TCH // 4, F], OUT_DT)
                    for r in range(RPC):
                        tp = ch * RPC + r
                        # e2[(th,s), d] = exp(score * -dist)
                        e2 = e2p.tile([128, D], MM_DT)
                        nc.scalar.activation(
                            out=e2[:], in_=ndist2[:],
                            func=mybir.ActivationFunctionType.Exp,
                            scale=score_t[:, tp : tp + 1],
                        )

                        # pm[128, 2048]: parity p-> cols p*1024; within parity:
                        # dblk = dh*4+dl -> col dh*512 + dl*65 (65 cols each)
                        pm = psump.tile([128, 2, 2, 512], F32, tag="pm")
                        for par in range(2):
                            p0 = par * S
                            for dh in range(2):
                                for dl in range(4):
                                    dblk = dh * 4 + dl
                                    nc.tensor.matmul(
                                        out=pm[:, par, dh, dl * 65 : dl * 65 + 65],
                                        lhsT=e2[p0 : p0 + S,
                                                dblk * 128 : (dblk + 1) * 128],
                                        rhs=x2m[p0 : p0 + S, tp, :],
                                        start=True, stop=True,
                                    )

                        # reciprocal of the 16 denominator columns
                        rc = small.tile([128, 2, 2, 4], F32, tag="rc")
                        nc.vector.reciprocal(
                            out=rc[:],
                            in_=pm[:, :, :, 64 : 64 + 3 * 65 + 1 : 65],
                        )

                        # drain + normalize: stage[.., 2r+par] = pm * rc
                        # (one rank-4 TT covers both parities)
                        tlh, tlo = (2 * r) // 8, (2 * r) % 8
                        out_ap = stage[:, :, :, tlh,
                                       tlo : tlo + 2, :].rearrange(
                            "p a c t e -> p t a c e")
                        rc_b = rc[:].unsqueeze(4).broadcast_to([128, 2, 2, 4, F])
                        rg = (b * NCH + ch) * RPC + r
                        if BOUNCE_PERIOD and rg % BOUNCE_PERIOD == (
                                BOUNCE_PERIOD - 1):
                            # psum -> SBUF on ScalarE, normalize on GpSimd
                            tmp = small.tile([128, 2, 2, 260], F32, tag="bnc")
                            nc.scalar.activation(
                                out=tmp[:], in_=pm[:, :, :, 0:260],
                                func=mybir.ActivationFunctionType.Copy,
                            )
                            nc.gpsimd.tensor_tensor(
                                out=out_ap,
                                in0=tmp[:].rearrange(
                                    "p a h (c e) -> p a h c e",
                                    c=4)[:, :, :, :, 0:F],
                                in1=rc_b,
                                op=mybir.AluOpType.mult,
                            )
                        else:
                            nc.vector.tensor_tensor(
                                out=out_ap,
                                in0=pm[:, :, :, 0 : 4 * 65].rearrange(
                                    "p a h (c e) -> p a h c e",
                                    c=4)[:, :, :, :, 0:F],
                                in1=rc_b,
                                op=mybir.AluOpType.mult,
                            )

                    # ---- DMA chunk out: one batched DMA per quarter-chunk
                    # covering all 8 d-blocks (1 MB each; per-descriptor runs
                    # of hp=512 contiguous elements per partition) ----------
                    hp = (TCH // 4) * F  # 512 elements per partition
                    for tlh in range(4):
                        nc.sync.dma_start(
                            out=dram_ap(
                                out_t,
                                ((b * NCH + ch) * 4 + tlh) * DBLK * 128 * hp,
                                [[hp, 128], [128 * hp, DBLK], [1, hp]],
                            ),
                            in_=stage[:, :, :, tlh, :, :],
                        )

    nc.compile()
    return nc


_NC_CACHE = None


def _get_nc():
    global _NC_CACHE
    if _NC_CACHE is None:
        _NC_CACHE = build_kernel()
    return _NC_CACHE


def kernel(X, dist, attention_weight, attention_bias):
    X = np.asarray(X, dtype=np.float32)                                # (S,B,F,T)
    dist_np = np.asarray(dist, dtype=np.float32).reshape(-1, S)        # (D,S)
    ndist_T = np.ascontiguousarray(-dist_np.T)                         # (S,D)
    w_np = np.ascontiguousarray(np.asarray(attention_weight, np.float32))
    bias_np = np.ascontiguousarray(
        np.asarray(attention_bias, np.float32).reshape(S, 1))
    # xp[th, s, b, tp, f] = X[s, b, f, 2*tp+th]
    xp_full = np.ascontiguousarray(
        X.reshape(S, B, F, TP, 2).transpose(4, 0, 1, 3, 2))

    nc = _get_nc()
    in_maps = []
    for c in range(NCORES):
        in_maps.append({
            "xp": np.ascontiguousarray(xp_full[:, :, c * BL : (c + 1) * BL]),
            "ndist_T": ndist_T,
            "w": w_np,
            "bias": bias_np,
        })
    res = bass_utils.run_bass_kernel_spmd(nc, in_maps, core_ids=list(range(NCORES)))
    # out_hw[b, ch, tlh, dblk, p, tlo, f]
    #   -> out[dblk*128+p, B-global, f, ch*TCH + tlh*8 + tlo]
    out = np.empty((D, B, F, T), dtype=np.float32)
    for c in range(NCORES):
        hw = res.results[c]["out_hw"]                # (BL,NCH,4,8,128,8,64)
        # -> (dblk, p, b, f, ch, tlh, tlo)
        out[:, c * BL : (c + 1) * BL] = (
            hw.astype(np.float32)
            .transpose(3, 4, 0, 6, 1, 2, 5)
            .reshape(D, BL, F, T)
        )
    return out.reshape(32, 32, B, F, T)



# revision 13
# speedup vs baseline: 2.4797x; 1.0843x over previous
"""Trainium2 Bass kernel for BroadcastingSelfAttention.

Reference computation (see problem):
    score(s,b,t) = softplus(sum_f X[s,b,f,t] * W[s,f] + bias[s])
    w(d,s,b,t)   = softmax_s(-score(s,b,t) * dist(d,s))
    out(d,b,f,t) = sum_s w(d,s,b,t) * X[s,b,f,t]

Shapes: S=64, B=16, F=64, T=96, D=1024 (= 32*32 target grid).

Sharding: B=16 split across 8 cores (2 batches per core). Each core reads its
X slice (3.1 MB) + full dist/params, writes its out slice (50 MB).

Per-core dataflow (per batch b, per t-pair):
  * e2[(th,s)=128p, d=1024] = exp(score(s, 2*tp+th) * (-dist(d,s)))  -- one ACT op
    (t-parity packs two t's into 128 partitions; dist pre-negated so the ACT
    per-partition `scale` operand carries +score)
  * 16 matmuls: stationary e2[s=64, d_blk=128 cols], moving [X[s,f,t] | ones]
    (N=65; fused denominator column), row-tiled by t-parity (tile_position 0/64)
    -> psum[d=128, f'=65] per (parity, d_blk)
  * reciprocal of the 16 denominator columns (DVE)
  * drain: TT multiply psum * recip (free-dim step-0 broadcast) -> staging SBUF
    in d-partition layout; t accumulates in stage; DMA out 128B-contiguous runs.
"""

import numpy as np

import concourse.bass as bass
import concourse.tile as tile
from concourse import bacc, mybir
from concourse import bass_utils

F32 = mybir.dt.float32
BF16 = mybir.dt.bfloat16

# Problem shapes (hardcoded per contract)
S = 64          # sources
B = 16          # total batch
NCORES = 8
BL = B // NCORES  # batches per core = 2
F = 64          # features
T = 96          # time
D = 1024        # flattened target grid 32*32
DBLK = D // 128  # 8 d-blocks of 128
TP = T // 2     # 48 t-pairs
TCH = 32        # t-chunk (stage tile holds 32 t values = 16 pairs)
NCH = T // TCH  # 3 chunks
RPC = TCH // 2  # 16 rounds (t-pairs) per chunk

FT = F * T            # 6144
SSTRIDE = BL * F * T  # x: s stride, 12288
OSTRIDE = BL * F * T  # out: d stride, 12288

# MM_DT: dtype of matmul operands (e2 weights + moving X). fp32 is exact;
# bf16 halves weight-load time (FWL) at ~0.4% relative error.
MM_DT = BF16
# OUT_DT: dtype of the staged/DMA'd output (host upcasts to f32). bf16 halves
# the dominant output DMA traffic at ~0.4% relative error.
OUT_DT = BF16
# Every Nth round bounces psum through SBUF via ScalarE-copy + GpSimd-
# normalize instead of the VectorE drain (0 = never). Offloads the DVE
# bottleneck without extra psum pressure. ScalarE is ~90% busy with the
# e2 exp, so only a small fraction of rounds can bounce.
BOUNCE_PERIOD = 5


def build_kernel():
    nc = bacc.Bacc("TRN2", target_bir_lowering=False, debug=False,
                   num_devices=NCORES)

    # xp[th, s, b, tp, f] = X[s, b, f, 2*tp+th]  (host pre-shuffled t-parity,
    # t-major so SBUF reads are f-contiguous)
    x_t = nc.dram_tensor("xp", (2, S, BL, TP, F), F32, kind="ExternalInput")
    # ndist_T[s, d] = -dist[d, s]  (host pre-transposed + negated)
    dist_t = nc.dram_tensor("ndist_T", (S, D), F32, kind="ExternalInput")
    w_t = nc.dram_tensor("w", (S, F), F32, kind="ExternalInput")
    bias_t = nc.dram_tensor("bias", (S, 1), F32, kind="ExternalInput")
    # Output in hardware-native layout: one fully-contiguous run per
    # partition per DMA (host un-permutes). Index: [b, ch, tlh, dblk, p, tlo, f]
    # -> out[dblk*128+p, b, f, ch*TCH + tlh*(TCH//4) + tlo].
    out_t = nc.dram_tensor("out_hw", (BL, NCH, 4, DBLK, 128, TCH // 4, F),
                           OUT_DT, kind="ExternalOutput")

    def dram_ap(t, offset, ap):
        base = t.ap()
        return bass.AP(tensor=base.tensor, offset=offset, ap=ap)

    with tile.TileContext(nc) as tc:
        with (
            tc.tile_pool(name="statics", bufs=1) as statics,
            tc.tile_pool(name="xin", bufs=2) as xin,
            tc.tile_pool(name="xmm", bufs=2) as xmm,
            tc.tile_pool(name="score", bufs=2) as scorep,
            tc.tile_pool(name="e2p", bufs=4) as e2p,
            tc.tile_pool(name="stage", bufs=3 if OUT_DT is BF16 else 2) as stagep,
            tc.tile_pool(name="small", bufs=4) as small,
            tc.tile_pool(name="psum", bufs=2, space="PSUM") as psump,
        ):
            # ---- static tiles -------------------------------------------------
            # ndist2[(th,s)=128p, d] = -dist(d,s), replicated across t-parity
            ndist2 = statics.tile([128, D], F32)
            for th in range(2):
                nc.gpsimd.dma_start(
                    out=ndist2[th * S : (th + 1) * S, :],
                    in_=dram_ap(dist_t, 0, [[D, S], [1, D]]),
                )

            # w2[(th,s), f] = W[s,f]
            w2 = statics.tile([128, F], F32)
            for th in range(2):
                nc.gpsimd.dma_start(
                    out=w2[th * S : (th + 1) * S, :],
                    in_=dram_ap(w_t, 0, [[F, S], [1, F]]),
                )
            # bias2[(th,s), 1]
            bias2 = statics.tile([128, 1], F32)
            for th in range(2):
                nc.gpsimd.dma_start(
                    out=bias2[th * S : (th + 1) * S, :],
                    in_=dram_ap(bias_t, 0, [[1, S], [0, 1]]),
                )

            for b in range(BL):
                # ---- x2[(th,s)=128p, tp=48, f'=65], f'==64 is the ones column
                x2 = xin.tile([128, TP, F + 1], F32)
                for th in range(2):
                    # parity halves on different DMA paths (SWDGE + HWDGE);
                    # b=0 th=0 rides the idle ACT HWDGE queue (startup path)
                    eng = (nc.scalar if b == 0 else nc.gpsimd) if th == 0 else nc.sync
                    eng.dma_start(
                        out=x2[th * S : (th + 1) * S, :, 0:F],
                        in_=dram_ap(
                            x_t, th * (S * BL * F * TP) + b * (F * TP),
                            [[BL * F * TP, S], [F, TP], [1, F]],
                        ),
                    )
                nc.vector.memset(x2[:, :, F : F + 1], 1.0)

                if MM_DT is F32:
                    x2m = x2
                else:
                    # cast per t-chunk so chunk 0's rounds start after ~1/3
                    # of the cast; GpSimd is otherwise idle here
                    x2m = xmm.tile([128, TP, F + 1], MM_DT)
                    for ch in range(NCH):
                        sl = slice(ch * RPC, (ch + 1) * RPC)
                        nc.gpsimd.tensor_copy(x2m[:, sl, :], x2[:, sl, :])

                # ---- score_t[(th,s), tp] = softplus(sum_f x*w + bias) --------
                ztmp = scorep.tile([128, TP, F], F32, tag="ztmp")
                nc.vector.tensor_tensor(
                    out=ztmp[:],
                    in0=x2[:, :, 0:F],
                    in1=w2[:].unsqueeze(1).broadcast_to([128, TP, F]),
                    op=mybir.AluOpType.mult,
                )
                z = scorep.tile([128, TP], F32, tag="z")
                nc.vector.reduce_sum(out=z[:], in_=ztmp[:],
                                     axis=mybir.AxisListType.X)
                # softplus(z+bias) = ln(1 + exp(z+bias)); exp & ln share one
                # activation table (natural_log_exp_and_others), softplus does
                # not exist on cayman hardware tables.
                ez = scorep.tile([128, TP], F32, tag="ez")
                nc.scalar.activation(
                    out=ez[:], in_=z[:],
                    func=mybir.ActivationFunctionType.Exp,
                    bias=bias2[:, 0:1], scale=1.0,
                )
                nc.vector.tensor_scalar_add(ez[:], ez[:], 1.0)
                score_t = scorep.tile([128, TP], F32, tag="score")
                nc.scalar.activation(
                    out=score_t[:], in_=ez[:],
                    func=mybir.ActivationFunctionType.Ln,
                )

                for ch in range(NCH):
                    # stage[(d%128)=128p, dh=2, dl=4, tlh=4, tlo=8, f=64]
                    # (t_local = tlh*8 + tlo; f innermost so drains write
                    # contiguous 128B runs; the tlh split lets early quarter-
                    # chunks DMA out while later rounds are still draining)
                    stage = stagep.tile([128, 2, 4, 4, TCH // 4, F], OUT_DT)
                    for r in range(RPC):
                        tp = ch * RPC + r
                        # e2[(th,s), d] = exp(score * -dist)
                        e2 = e2p.tile([128, D], MM_DT)
                        nc.scalar.activation(
                            out=e2[:], in_=ndist2[:],
                            func=mybir.ActivationFunctionType.Exp,
                            scale=score_t[:, tp : tp + 1],
                        )

                        # pm[128, 2048]: parity p-> cols p*1024; within parity:
                        # dblk = dh*4+dl -> col dh*512 + dl*65 (65 cols each)
                        # Parity innermost: consecutive matmuls target
                        # alternating PE row-groups, so each LDWEIGHTS
                        # overlaps the other parity's in-flight MATMUL.
                        pm = psump.tile([128, 2, 2, 512], F32, tag="pm")
                        for dh in range(2):
                            for dl in range(4):
                                dblk = dh * 4 + dl
                                for par in range(2):
                                    p0 = par * S
                                    nc.tensor.matmul(
                                        out=pm[:, par, dh, dl * 65 : dl * 65 + 65],
                                        lhsT=e2[p0 : p0 + S,
                                                dblk * 128 : (dblk + 1) * 128],
                                        rhs=x2m[p0 : p0 + S, tp, :],
                                        start=True, stop=True,
                                    )

                        # reciprocal of the 16 denominator columns
                        rc = small.tile([128, 2, 2, 4], F32, tag="rc")
                        nc.vector.reciprocal(
                            out=rc[:],
                            in_=pm[:, :, :, 64 : 64 + 3 * 65 + 1 : 65],
                        )

                        # drain + normalize: stage[.., 2r+par] = pm * rc
                        # (one rank-4 TT covers both parities)
                        tlh, tlo = (2 * r) // 8, (2 * r) % 8
                        out_ap = stage[:, :, :, tlh,
                                       tlo : tlo + 2, :].rearrange(
                            "p a c t e -> p t a c e")
                        rc_b = rc[:].unsqueeze(4).broadcast_to([128, 2, 2, 4, F])
                        rg = (b * NCH + ch) * RPC + r
                        if BOUNCE_PERIOD and rg % BOUNCE_PERIOD == (
                                BOUNCE_PERIOD - 1):
                            # psum -> SBUF on ScalarE, normalize on GpSimd
                            tmp = small.tile([128, 2, 2, 260], F32, tag="bnc")
                            nc.scalar.activation(
                                out=tmp[:], in_=pm[:, :, :, 0:260],
                                func=mybir.ActivationFunctionType.Copy,
                            )
                            nc.gpsimd.tensor_tensor(
                                out=out_ap,
                                in0=tmp[:].rearrange(
                                    "p a h (c e) -> p a h c e",
                                    c=4)[:, :, :, :, 0:F],
                                in1=rc_b,
                                op=mybir.AluOpType.mult,
                            )
                        else:
                            nc.vector.tensor_tensor(
                                out=out_ap,
                                in0=pm[:, :, :, 0 : 4 * 65].rearrange(
                                    "p a h (c e) -> p a h c e",
                                    c=4)[:, :, :, :, 0:F],
                                in1=rc_b,
                                op=mybir.AluOpType.mult,
                            )

                    # ---- DMA chunk out: one batched DMA per quarter-chunk
                    # covering all 8 d-blocks (1 MB each; per-descriptor runs
                    # of hp=512 contiguous elements per partition) ----------
                    hp = (TCH // 4) * F  # 512 elements per partition
                    for tlh in range(4):
                        nc.sync.dma_start(
                            out=dram_ap(
                                out_t,
                                ((b * NCH + ch) * 4 + tlh) * DBLK * 128 * hp,
                                [[hp, 128], [128 * hp, DBLK], [1, hp]],
                            ),
                            in_=stage[:, :, :, tlh, :, :],
                        )

    nc.compile()
    return nc


_NC_CACHE = None


def _get_nc():
    global _NC_CACHE
    if _NC_CACHE is None:
        _NC_CACHE = build_kernel()
    return _NC_CACHE


def kernel(X, dist, attention_weight, attention_bias):
    X = np.asarray(X, dtype=np.float32)                                # (S,B,F,T)
    dist_np = np.asarray(dist, dtype=np.float32).reshape(-1, S)        # (D,S)
    ndist_T = np.ascontiguousarray(-dist_np.T)                         # (S,D)
    w_np = np.ascontiguousarray(np.asarray(attention_weight, np.float32))
    bias_np = np.ascontiguousarray(
        np.asarray(attention_bias, np.float32).reshape(S, 1))
    # xp[th, s, b, tp, f] = X[s, b, f, 2*tp+th]
    xp_full = np.ascontiguousarray(
        X.reshape(S, B, F, TP, 2).transpose(4, 0, 1, 3, 2))

    nc = _get_nc()
    in_maps = []
    for c in range(NCORES):
        in_maps.append({
            "xp": np.ascontiguousarray(xp_full[:, :, c * BL : (c + 1) * BL]),
            "ndist_T": ndist_T,
            "w": w_np,
            "bias": bias_np,
        })
    res = bass_utils.run_bass_kernel_spmd(nc, in_maps, core_ids=list(range(NCORES)))
    # out_hw[b, ch, tlh, dblk, p, tlo, f]
    #   -> out[dblk*128+p, B-global, f, ch*TCH + tlh*8 + tlo]
    out = np.empty((D, B, F, T), dtype=np.float32)
    for c in range(NCORES):
        hw = res.results[c]["out_hw"]                # (BL,NCH,4,8,128,8,64)
        # -> (dblk, p, b, f, ch, tlh, tlo)
        out[:, c * BL : (c + 1) * BL] = (
            hw.astype(np.float32)
            .transpose(3, 4, 0, 6, 1, 2, 5)
            .reshape(D, BL, F, T)
        )
    return out.reshape(32, 32, B, F, T)



# revision 21
# speedup vs baseline: 2.5401x; 1.0243x over previous
"""Trainium2 Bass kernel for BroadcastingSelfAttention.

Reference computation (see problem):
    score(s,b,t) = softplus(sum_f X[s,b,f,t] * W[s,f] + bias[s])
    w(d,s,b,t)   = softmax_s(-score(s,b,t) * dist(d,s))
    out(d,b,f,t) = sum_s w(d,s,b,t) * X[s,b,f,t]

Shapes: S=64, B=16, F=64, T=96, D=1024 (= 32*32 target grid).

Sharding: B=16 split across 8 cores (2 batches per core). Each core reads its
X slice (3.1 MB) + full dist/params, writes its out slice (50 MB).

Per-core dataflow (per batch b, per t-pair):
  * e2[(th,s)=128p, d=1024] = exp(score(s, 2*tp+th) * (-dist(d,s)))  -- one ACT op
    (t-parity packs two t's into 128 partitions; dist pre-negated so the ACT
    per-partition `scale` operand carries +score)
  * 16 matmuls: stationary e2[s=64, d_blk=128 cols], moving [X[s,f,t] | ones]
    (N=65; fused denominator column), row-tiled by t-parity (tile_position 0/64)
    -> psum[d=128, f'=65] per (parity, d_blk)
  * reciprocal of the 16 denominator columns (DVE)
  * drain: TT multiply psum * recip (free-dim step-0 broadcast) -> staging SBUF
    in d-partition layout; t accumulates in stage; DMA out 128B-contiguous runs.
"""

import numpy as np

import concourse.bass as bass
import concourse.tile as tile
from concourse import bacc, mybir
from concourse import bass_utils

F32 = mybir.dt.float32
BF16 = mybir.dt.bfloat16

# Problem shapes (hardcoded per contract)
S = 64          # sources
B = 16          # total batch
NCORES = 8
BL = B // NCORES  # batches per core = 2
F = 64          # features
T = 96          # time
D = 1024        # flattened target grid 32*32
DBLK = D // 128  # 8 d-blocks of 128
TP = T // 2     # 48 t-pairs
TCH = 32        # t-chunk (stage tile holds 32 t values = 16 pairs)
NCH = T // TCH  # 3 chunks
RPC = TCH // 2  # 16 rounds (t-pairs) per chunk

FT = F * T            # 6144
SSTRIDE = BL * F * T  # x: s stride, 12288
OSTRIDE = BL * F * T  # out: d stride, 12288

# MM_DT: dtype of matmul operands (e2 weights + moving X). fp32 is exact;
# bf16 halves weight-load time (FWL) at ~0.4% relative error.
MM_DT = BF16
# OUT_DT: dtype of the staged/DMA'd output (host upcasts to f32). bf16 halves
# the dominant output DMA traffic at ~0.4% relative error.
OUT_DT = BF16
# Every Nth round bounces psum through SBUF via ScalarE-copy + GpSimd-
# normalize instead of the VectorE drain (0 = never). Equilibrium between
# DVE (TT 1.22 + recip 0.25 us/round) and ScalarE (exp 1.21 + 1.1/bounce):
# bounce ~1 round in 8.
BOUNCE_PERIOD = 8


def build_kernel():
    nc = bacc.Bacc("TRN2", target_bir_lowering=False, debug=False,
                   num_devices=NCORES)

    # xp[th, s, b, tp, f] = X[s, b, f, 2*tp+th]  (host pre-shuffled t-parity,
    # t-major so SBUF reads are f-contiguous)
    x_t = nc.dram_tensor("xp", (2, S, BL, TP, F), F32, kind="ExternalInput")
    # ndist_T[s, d] = -dist[d, s]  (host pre-transposed + negated)
    dist_t = nc.dram_tensor("ndist_T", (S, D), F32, kind="ExternalInput")
    w_t = nc.dram_tensor("w", (S, F), F32, kind="ExternalInput")
    bias_t = nc.dram_tensor("bias", (S, 1), F32, kind="ExternalInput")
    # Output in hardware-native layout: one fully-contiguous run per
    # partition per DMA (host un-permutes). Index: [b, ch, tlh, dblk, p, tlo, f]
    # -> out[dblk*128+p, b, f, ch*TCH + tlh*(TCH//4) + tlo].
    out_t = nc.dram_tensor("out_hw", (BL, NCH, 4, DBLK, 128, TCH // 4, F),
                           OUT_DT, kind="ExternalOutput")

    def dram_ap(t, offset, ap):
        base = t.ap()
        return bass.AP(tensor=base.tensor, offset=offset, ap=ap)

    with tile.TileContext(nc) as tc:
        with (
            tc.tile_pool(name="statics", bufs=1) as statics,
            tc.tile_pool(name="xin", bufs=2) as xin,
            tc.tile_pool(name="xmm", bufs=2) as xmm,
            tc.tile_pool(name="score", bufs=2) as scorep,
            tc.tile_pool(name="e2p", bufs=4) as e2p,
            tc.tile_pool(name="stage", bufs=3 if OUT_DT is BF16 else 2) as stagep,
            tc.tile_pool(name="small", bufs=4) as small,
            tc.tile_pool(name="psum", bufs=2, space="PSUM") as psump,
        ):
            # ---- static tiles (spread across DGE queues so they load in
            # parallel; w2/bias2 gate the score chain, ndist2 gates exp) ----
            # ndist2[(th,s)=128p, d] = -dist(d,s), replicated across t-parity
            ndist2 = statics.tile([128, D], F32)
            for th, eng in ((0, nc.gpsimd), (1, nc.sync)):
                eng.dma_start(
                    out=ndist2[th * S : (th + 1) * S, :],
                    in_=dram_ap(dist_t, 0, [[D, S], [1, D]]),
                )

            # w2[(th,s), f] = W[s,f]
            w2 = statics.tile([128, F], F32)
            for th, eng in ((0, nc.sync), (1, nc.scalar)):
                eng.dma_start(
                    out=w2[th * S : (th + 1) * S, :],
                    in_=dram_ap(w_t, 0, [[F, S], [1, F]]),
                )
            # bias2[(th,s), 1]
            bias2 = statics.tile([128, 1], F32)
            for th in range(2):
                nc.scalar.dma_start(
                    out=bias2[th * S : (th + 1) * S, :],
                    in_=dram_ap(bias_t, 0, [[1, S], [0, 1]]),
                )

            for b in range(BL):
                # ---- x2[(th,s)=128p, tp=48, f'=65], f'==64 is the ones column
                # (whole-tile memset first: packed/contiguous, and it leaves
                # the ones column set; the DMA then overwrites cols 0:F with
                # one 12KB-contiguous descriptor per partition)
                x2 = xin.tile([128, TP, F + 1], F32)
                nc.vector.memset(x2[:], 1.0)
                for th in range(2):
                    # parity halves on different DMA paths (SWDGE + HWDGE);
                    # b=0 th=0 rides the idle ACT HWDGE queue (startup path)
                    eng = (nc.scalar if b == 0 else nc.gpsimd) if th == 0 else nc.sync
                    eng.dma_start(
                        out=x2[th * S : (th + 1) * S, :, 0:F],
                        in_=dram_ap(
                            x_t, th * (S * BL * F * TP) + b * (F * TP),
                            [[BL * F * TP, S], [1, TP * F]],
                        ),
                    )

                if MM_DT is F32:
                    x2m = x2
                else:
                    # cast per t-chunk so chunk 0's rounds start after ~1/3
                    # of the cast; GpSimd is otherwise idle here
                    x2m = xmm.tile([128, TP, F + 1], MM_DT)
                    for ch in range(NCH):
                        sl = slice(ch * RPC, (ch + 1) * RPC)
                        nc.gpsimd.tensor_copy(x2m[:, sl, :], x2[:, sl, :])

                # ---- score_t[(th,s), tp] = softplus(sum_f x*w + bias) --------
                ztmp = scorep.tile([128, TP, F], F32, tag="ztmp")
                nc.vector.tensor_tensor(
                    out=ztmp[:],
                    in0=x2[:, :, 0:F],
                    in1=w2[:].unsqueeze(1).broadcast_to([128, TP, F]),
                    op=mybir.AluOpType.mult,
                )
                z = scorep.tile([128, TP], F32, tag="z")
                nc.vector.reduce_sum(out=z[:], in_=ztmp[:],
                                     axis=mybir.AxisListType.X)
                # softplus(z+bias) = ln(1 + exp(z+bias)); exp & ln share one
                # activation table (natural_log_exp_and_others), softplus does
                # not exist on cayman hardware tables.
                ez = scorep.tile([128, TP], F32, tag="ez")
                nc.scalar.activation(
                    out=ez[:], in_=z[:],
                    func=mybir.ActivationFunctionType.Exp,
                    bias=bias2[:, 0:1], scale=1.0,
                )
                nc.vector.tensor_scalar_add(ez[:], ez[:], 1.0)
                score_t = scorep.tile([128, TP], F32, tag="score")
                nc.scalar.activation(
                    out=score_t[:], in_=ez[:],
                    func=mybir.ActivationFunctionType.Ln,
                )

                for ch in range(NCH):
                    # stage[(d%128)=128p, dh=2, dl=4, tlh=4, tlo=8, f=64]
                    # (t_local = tlh*8 + tlo; f innermost so drains write
                    # contiguous 128B runs; the tlh split lets early quarter-
                    # chunks DMA out while later rounds are still draining)
                    stage = stagep.tile([128, 2, 4, 4, TCH // 4, F], OUT_DT)
                    for r in range(RPC):
                        tp = ch * RPC + r
                        # e2[(th,s), d] = exp(score * -dist)
                        e2 = e2p.tile([128, D], MM_DT)
                        nc.scalar.activation(
                            out=e2[:], in_=ndist2[:],
                            func=mybir.ActivationFunctionType.Exp,
                            scale=score_t[:, tp : tp + 1],
                        )

                        # pm[128, 2048]: parity p-> cols p*1024; within parity:
                        # dblk = dh*4+dl -> col dh*512 + dl*65 (65 cols each)
                        # Parity innermost: consecutive matmuls target
                        # alternating PE row-groups, so each LDWEIGHTS
                        # overlaps the other parity's in-flight MATMUL.
                        pm = psump.tile([128, 2, 2, 512], F32, tag="pm")
                        for dh in range(2):
                            for dl in range(4):
                                dblk = dh * 4 + dl
                                for par in range(2):
                                    p0 = par * S
                                    nc.tensor.matmul(
                                        out=pm[:, par, dh, dl * 65 : dl * 65 + 65],
                                        lhsT=e2[p0 : p0 + S,
                                                dblk * 128 : (dblk + 1) * 128],
                                        rhs=x2m[p0 : p0 + S, tp, :],
                                        start=True, stop=True,
                                    )

                        # reciprocal of the 16 denominator columns (the HW
                        # allows only ONE psum operand per instruction, so a
                        # fused psum/psum divide is not possible)
                        rc = small.tile([128, 2, 2, 4], F32, tag="rc")
                        nc.vector.reciprocal(
                            out=rc[:],
                            in_=pm[:, :, :, 64 : 64 + 3 * 65 + 1 : 65],
                        )

                        # drain + normalize: stage[.., 2r+par] = pm * rc
                        # (one rank-4 TT covers both parities)
                        tlh, tlo = (2 * r) // 8, (2 * r) % 8
                        out_ap = stage[:, :, :, tlh,
                                       tlo : tlo + 2, :].rearrange(
                            "p a c t e -> p t a c e")
                        rc_b = rc[:].unsqueeze(4).broadcast_to([128, 2, 2, 4, F])
                        rg = (b * NCH + ch) * RPC + r
                        if BOUNCE_PERIOD and rg % BOUNCE_PERIOD == (
                                BOUNCE_PERIOD - 1):
                            # psum -> SBUF on ScalarE, normalize on GpSimd
                            tmp = small.tile([128, 2, 2, 260], F32, tag="bnc")
                            nc.scalar.activation(
                                out=tmp[:], in_=pm[:, :, :, 0:260],
                                func=mybir.ActivationFunctionType.Copy,
                            )
                            nc.gpsimd.tensor_tensor(
                                out=out_ap,
                                in0=tmp[:].rearrange(
                                    "p a h (c e) -> p a h c e",
                                    c=4)[:, :, :, :, 0:F],
                                in1=rc_b,
                                op=mybir.AluOpType.mult,
                            )
                        else:
                            nc.vector.tensor_tensor(
                                out=out_ap,
                                in0=pm[:, :, :, 0 : 4 * 65].rearrange(
                                    "p a h (c e) -> p a h c e",
                                    c=4)[:, :, :, :, 0:F],
                                in1=rc_b,
                                op=mybir.AluOpType.mult,
                            )

                    # ---- DMA chunk out: one batched DMA per quarter-chunk
                    # covering all 8 d-blocks (1 MB each; per-descriptor runs
                    # of hp=512 contiguous elements per partition) ----------
                    hp = (TCH // 4) * F  # 512 elements per partition
                    for tlh in range(4):
                        nc.sync.dma_start(
                            out=dram_ap(
                                out_t,
                                ((b * NCH + ch) * 4 + tlh) * DBLK * 128 * hp,
                                [[hp, 128], [128 * hp, DBLK], [1, hp]],
                            ),
                            in_=stage[:, :, :, tlh, :, :],
                        )

    nc.compile()
    return nc


_NC_CACHE = None


def _get_nc():
    global _NC_CACHE
    if _NC_CACHE is None:
        _NC_CACHE = build_kernel()
    return _NC_CACHE


def kernel(X, dist, attention_weight, attention_bias):
    X = np.asarray(X, dtype=np.float32)                                # (S,B,F,T)
    dist_np = np.asarray(dist, dtype=np.float32).reshape(-1, S)        # (D,S)
    ndist_T = np.ascontiguousarray(-dist_np.T)                         # (S,D)
    w_np = np.ascontiguousarray(np.asarray(attention_weight, np.float32))
    bias_np = np.ascontiguousarray(
        np.asarray(attention_bias, np.float32).reshape(S, 1))
    # xp[th, s, b, tp, f] = X[s, b, f, 2*tp+th]
    xp_full = np.ascontiguousarray(
        X.reshape(S, B, F, TP, 2).transpose(4, 0, 1, 3, 2))

    nc = _get_nc()
    in_maps = []
    for c in range(NCORES):
        in_maps.append({
            "xp": np.ascontiguousarray(xp_full[:, :, c * BL : (c + 1) * BL]),
            "ndist_T": ndist_T,
            "w": w_np,
            "bias": bias_np,
        })
    res = bass_utils.run_bass_kernel_spmd(nc, in_maps, core_ids=list(range(NCORES)))
    # out_hw[b, ch, tlh, dblk, p, tlo, f]
    #   -> out[dblk*128+p, B-global, f, ch*TCH + tlh*8 + tlo]
    out = np.empty((D, B, F, T), dtype=np.float32)
    for c in range(NCORES):
        hw = res.results[c]["out_hw"]                # (BL,NCH,4,8,128,8,64)
        # -> (dblk, p, b, f, ch, tlh, tlo)
        out[:, c * BL : (c + 1) * BL] = (
            hw.astype(np.float32)
            .transpose(3, 4, 0, 6, 1, 2, 5)
            .reshape(D, BL, F, T)
        )
    return out.reshape(32, 32, B, F, T)

